# revision 9
# baseline (speedup 1.0000x reference)
"""Batched compressed linear: y = x @ (w_q * scale).T + bias on 8 TRN2 cores.

Sharding: column-parallel over out_features (16384 -> 8 x 2048).
Each core computes y_shard[8192, 2048] = x[8192, 4096] @ wT_shard + bias_shard.

Measured (this box, slope bench): 1665-1800 us across runs (run-to-run
variance ~±5% for the same NEFF), fro rel err 1.59482e-2 (deterministic,
seeded inputs; gate 2e-2). Session baseline was 2284 us / 2.27e-3.

Design (HW-measured on trn2 via microbench.py):
  - Main loop is pair-interleaved ("pairk"): consecutive matmuls ALTERNATE
    the stationary xT slice between k=2p and 2p+1 while rotating the 4
    psum banks. Measured 249.9 ns/MM standalone (vs 264.7-287.2 for the
    quad order that reuses one stationary 4x: back-to-back LDWEIGHTS of
    the SAME address is a penalty, not a saving; same-bank psum
    back-to-back costs ~19 ns/MM). In-kernel the pairk stream runs at
    ~220 ns/MM (N=512 compute floor is 213.3 ns).
  - NF8 of the 32 k-blocks run as fp8e4m3 DoubleRow matmuls (2 k-planes
    per MM, ~300 ns vs 2x220 bf16): x8 = fp8(bf16 x), w8 = fp8(bf16
    w*scale), single digit each. This trades accuracy for time:
    fro rel err ~= sqrt(NF8/32)*3.6% (deterministic, seeded inputs);
    NF8=6 measures ~1.65e-2 against the 2e-2 gate. DR MMs are placed
    contiguously at the head of each m-tile (mixed_c ordering measured
    1.06us/rep cheaper than interleaving them among the bf16 groups).
    Full fp8 digit schemes were measured and rejected: exact int8-weight
    arithmetic needs >= 3 fp8 digit-planes per k-block = 444 ns vs
    bf16's 2x220.
  - scale is folded into w at preproc (w_sc = bf16(w_q)*scale), so the
    psum evict is a single DVE bias-add.
  - w preproc: int32 staged [128,1024] chunks (loads split across the
    sync and gpsimd rings), DVE tensor_scalar_mul to bf16, SBUF->SBUF
    xbar transpose per 128-row strip into k-major banks: blocks
    NF8..31 into resident bf16 banks, blocks 0..NF8-1 into a bf16
    scratch then DVE-cast to the resident fp8 banks (the xbar transpose
    only supports 2-byte dtypes).
  - x: on-chip path (no DRAM round trip): fp32 [128,1024] chunk loads
    (gpsimd), DVE cast to a bf16 strip, SBUF->SBUF xbar transpose into
    k-major xT tiles; blocks 0..NF8-1 then DVE-cast to fp8 xT8. mt0's
    loads lead the gpsimd ring; startup transposes ride sync, steady
    -state ones the scalar ring behind the 16 one-time w transposes.
"""

import sys

if "/opt/trn_rl_repo" not in sys.path:
    sys.path.insert(0, "/opt/trn_rl_repo")

import numpy as np

B, S, IN_F, OUT_F = 4, 2048, 4096, 16384
NCORES = 8
O_SHARD = OUT_F // NCORES  # 2048
M_FULL = B * S  # 8192
NF8 = 6  # k-blocks (x128) computed in fp8 DoubleRow; must be even


def build_kernel_v4(nc, tc, M, K, O):
    import concourse.mybir as mybir

    f32 = mybir.dt.float32
    bf16 = mybir.dt.bfloat16
    f8 = mybir.dt.float8e4
    i32 = mybir.dt.int32
    DR = mybir.MatmulPerfMode.DoubleRow

    assert M % 128 == 0 and K % 1024 == 0 and O % 512 == 0
    KT = K // 128  # 32 contraction tiles
    MT = M // 128  # 64 m tiles
    NB = O // 512  # 4 psum-bank columns
    OT = O // 128  # 16 w row-chunks
    WCH = 1024  # w free-dim chunk for staging
    NWQ = K // WCH  # 4 chunks per ot
    XCH = 1024
    NXQ = K // XCH
    NBF = KT - NF8  # bf16 k-blocks
    assert NF8 % 2 == 0 and NBF % 2 == 0

    x_d = nc.dram_tensor("x", [M, K], f32, kind="ExternalInput").ap()
    w_d = nc.dram_tensor("w_q", [O, K], i32, kind="ExternalInput").ap()
    scale_d = nc.dram_tensor("scale", [1], f32, kind="ExternalInput").ap()
    bias_d = nc.dram_tensor("bias", [O], f32, kind="ExternalInput").ap()
    y_d = nc.dram_tensor("y", [M, O], f32, kind="ExternalOutput").ap()

    from contextlib import ExitStack

    ctx = ExitStack()
    tc_pool = lambda **kw: ctx.enter_context(tc.tile_pool(**kw))

    consts = tc_pool(name="consts", bufs=1)
    wq_pool = tc_pool(name="wq", bufs=1)
    wtmp_pool = tc_pool(name="wtmp", bufs=2)
    xs_pool = tc_pool(name="xs", bufs=1)  # 2 tags -> still double-buffered
    xbf_pool = tc_pool(name="xbf", bufs=1)
    xt_pool = tc_pool(name="xt", bufs=4)
    out_pool = tc_pool(name="outsb", bufs=4)
    psum_pool = tc_pool(name="psum", bufs=2, space="PSUM")

    # ---- constants ----
    scale128 = consts.tile([128, 1], f32, tag="scale128")
    nc.sync.dma_start(scale128[:], scale_d[None, :].partition_broadcast(128))

    # ---- x pipeline: fp32 chunk loads -> DVE cast -> xbar transpose ----
    def emit_xpipe(mt, load_eng, t_eng):
        xbf = xbf_pool.tile([128, K], bf16, tag="xbf", name=f"xbf{mt}")
        for q in range(NXQ):
            c0 = q * XCH
            xs = xs_pool.tile([128, XCH], f32, tag=f"xs{q % 2}", name=f"xs{mt}_{q}")
            load_eng.dma_start(xs[:], x_d[mt * 128 : (mt + 1) * 128, c0 : c0 + XCH])
            nc.vector.tensor_scalar_mul(xbf[:, c0 : c0 + XCH], xs[:], 1.0)
        xT = xt_pool.tile([128, KT, 128], bf16, tag="xT", name=f"xT{mt}")
        t_eng.dma_start(xT[:], xbf[:], transpose=True)
        if NF8:
            xT8 = xt_pool.tile([128, NF8, 128], f8, tag="xT8", name=f"xT8_{mt}")
            nc.vector.tensor_scalar_mul(xT8[:], xT[:, 0:NF8, :], 1.0)
        else:
            xT8 = None
        return xT, xT8

    # mt0 x loads lead the gpsimd ring (before the w q2/q3 chunks)
    xt_q = [emit_xpipe(0, nc.gpsimd, nc.sync)]

    # ---- w preproc ----
    # bf16 banks hold k-blocks NF8..31: wT_banks[b][p, kk, j] =
    #   w_sc[b*512 + j, (kk+NF8)*128 + p]
    # fp8 banks hold k-blocks 0..NF8-1: wT8_banks[b][p, k, j]
    wT_banks = [
        consts.tile([128, NBF, 512], bf16, tag=f"wT{b}", name=f"wT{b}")
        for b in range(NB)
    ]
    wT8_banks = [
        consts.tile([128, NF8, 512], f8, tag=f"wT8{b}", name=f"wT8{b}")
        for b in range(NB)
    ] if NF8 else []
    for ot in range(OT):
        r0 = ot * 128
        w_sc4 = wq_pool.tile([128, K], bf16, tag="wsc4", name=f"wsc{ot}")
        for q in range(NWQ):
            c0 = q * WCH
            w_stage = wq_pool.tile(
                [128, WCH], i32, tag=f"wstage{q % 2}", name=f"wst{ot}_{q}"
            )
            # split the 33.5MB of w loads across the sync + gpsimd rings
            eng = nc.sync if q < 2 else nc.gpsimd
            eng.dma_start(w_stage[:], w_d[r0 : r0 + 128, c0 : c0 + WCH])
            nc.vector.tensor_scalar_mul(
                w_sc4[:, c0 : c0 + WCH], w_stage[:], scale128[:, 0:1]
            )
        b, col = ot // 4, (ot % 4) * 128
        nc.scalar.dma_start(
            wT_banks[b][:, :, col : col + 128],
            w_sc4[:, NF8 * 128 :],
            transpose=True,
        )
        if NF8:
            wtmp = wtmp_pool.tile([128, NF8, 128], bf16, tag="wtmp", name=f"wtm{ot}")
            nc.scalar.dma_start(wtmp[:], w_sc4[:, 0 : NF8 * 128], transpose=True)
            nc.vector.tensor_scalar_mul(
                wT8_banks[b][:, :, col : col + 128], wtmp[:], 1.0
            )
        if ot == 0:
            # mt1 x loads follow bank0's first strip on gpsimd; transpose on
            # sync (scalar carries the w transposes)
            xt_q.append(emit_xpipe(1, nc.gpsimd, nc.sync))
        elif ot == 7:
            xt_q.append(emit_xpipe(2, nc.gpsimd, nc.sync))

    bias_bcast = consts.tile([128, O], f32, tag="bias_bcast")
    nc.sync.dma_start(bias_bcast[:], bias_d[None, :].partition_broadcast(128))

    # ---- main loop ----
    for mt in range(MT):
        if xt_q:
            xT, xT8 = xt_q.pop(0)
        else:
            xT, xT8 = emit_xpipe(mt, nc.gpsimd, nc.scalar)
        if mt + 3 < MT:
            xt_q.append(emit_xpipe(mt + 3, nc.gpsimd, nc.scalar))
        banks = [
            psum_pool.tile([128, 512], f32, tag=f"ps{b}", name=f"ps{mt}_{b}")
            for b in range(NB)
        ]
        if mt < 2:
            # staged start: bank-sequential so mt0 begins once bank 0's w
            # strips are ready, instead of waiting for all 16
            for ob in range(NB):
                for t in range(NF8 // 2):
                    nc.tensor.matmul(
                        banks[ob][:],
                        xT8[:, 2 * t : 2 * t + 2, :],
                        wT8_banks[ob][:, 2 * t : 2 * t + 2, :],
                        start=(t == 0),
                        stop=False,
                        perf_mode=DR,
                    )
                for kk in range(NBF):
                    nc.tensor.matmul(
                        banks[ob][:],
                        xT[:, NF8 + kk, :],
                        wT_banks[ob][:, kk, :],
                        start=(NF8 == 0 and kk == 0),
                        stop=(kk == NBF - 1),
                    )
        else:
            # fp8 DR head (contiguous, t-outer: same-stationary x4 measured
            # faster than alternating for DR), then bf16 pairk over NF8..31
            for t in range(NF8 // 2):
                for ob in range(NB):
                    nc.tensor.matmul(
                        banks[ob][:],
                        xT8[:, 2 * t : 2 * t + 2, :],
                        wT8_banks[ob][:, 2 * t : 2 * t + 2, :],
                        start=(t == 0),
                        stop=False,
                        perf_mode=DR,
                    )
            for p in range(NBF // 2):
                for half in range(2):
                    for ob in range(NB):
                        kk = 2 * p + ((ob + half) % 2)
                        nc.tensor.matmul(
                            banks[ob][:],
                            xT[:, NF8 + kk, :],
                            wT_banks[ob][:, kk, :],
                            start=(NF8 == 0 and p == 0 and half == 0),
                            stop=(p == NBF // 2 - 1 and half == 1),
                        )
        for ob in range(NB):
            out_sb = out_pool.tile([128, 512], f32, tag="out", name=f"out{mt}_{ob}")
            nc.vector.tensor_add(
                out_sb[:], banks[ob][:], bias_bcast[:, ob * 512 : (ob + 1) * 512]
            )
            nc.sync.dma_start(
                y_d[mt * 128 : (mt + 1) * 128, ob * 512 : (ob + 1) * 512], out_sb[:]
            )

    ctx.close()


_CACHED_NC = None
LAST_RESULT = None


def _build_full_nc():
    global _CACHED_NC
    if _CACHED_NC is not None:
        return _CACHED_NC
    import concourse.tile as tile
    from concourse import bacc

    nc = bacc.Bacc(
        "TRN2",
        target_bir_lowering=False,
        debug=False,
        num_devices=NCORES,
    )
    with tile.TileContext(nc) as tc:
        build_kernel_v4(nc, tc, M_FULL, IN_F, O_SHARD)
    nc.compile()
    _CACHED_NC = nc
    return nc


def kernel(x, w_q, scale, bias):
    """Full inputs in, full output out. Shards w_q/bias over 8 cores."""
    from concourse.bass_utils import run_bass_kernel_spmd

    nc = _build_full_nc()

    x2 = np.ascontiguousarray(np.asarray(x, dtype=np.float32).reshape(M_FULL, IN_F))
    w2 = np.ascontiguousarray(np.asarray(w_q, dtype=np.int32))
    sc = np.asarray(scale, dtype=np.float32).reshape(1)
    bi = np.asarray(bias, dtype=np.float32)

    in_maps = []
    for c in range(NCORES):
        o0 = c * O_SHARD
        in_maps.append(
            {
                "x": x2,
                "w_q": np.ascontiguousarray(w2[o0 : o0 + O_SHARD]),
                "scale": sc,
                "bias": np.ascontiguousarray(bi[o0 : o0 + O_SHARD]),
            }
        )

    res = run_bass_kernel_spmd(nc, in_maps, core_ids=list(range(NCORES)))
    global LAST_RESULT
    LAST_RESULT = res
    shards = [res.results[c]["y"] for c in range(NCORES)]
    y = np.concatenate(shards, axis=1).reshape(B, S, OUT_F)
    return y.astype(np.float32)


# revision 10
# speedup vs baseline: 1.7973x; 1.7973x over previous
"""Batched compressed linear: y = x @ (w_q * scale).T + bias on 8 TRN2 cores.

Sharding: column-parallel over out_features (16384 -> 8 x 2048).
Each core computes y_shard[8192, 2048] = x[8192, 4096] @ wT_shard + bias_shard.

Measured (this box, slope bench): 1665-1800 us across runs (run-to-run
variance ~±5% for the same NEFF), fro rel err 1.59482e-2 (deterministic,
seeded inputs; gate 2e-2). Session baseline was 2284 us / 2.27e-3.

Design (HW-measured on trn2 via microbench.py):
  - Main loop is pair-interleaved ("pairk"): consecutive matmuls ALTERNATE
    the stationary xT slice between k=2p and 2p+1 while rotating the 4
    psum banks. Measured 249.9 ns/MM standalone (vs 264.7-287.2 for the
    quad order that reuses one stationary 4x: back-to-back LDWEIGHTS of
    the SAME address is a penalty, not a saving; same-bank psum
    back-to-back costs ~19 ns/MM). In-kernel the pairk stream runs at
    ~220 ns/MM (N=512 compute floor is 213.3 ns).
  - NF8 of the 32 k-blocks run as fp8e4m3 DoubleRow matmuls (2 k-planes
    per MM, ~300 ns vs 2x220 bf16): x8 = fp8(bf16 x), w8 = fp8(bf16
    w*scale), single digit each. This trades accuracy for time:
    fro rel err ~= sqrt(NF8/32)*3.6% (deterministic, seeded inputs);
    NF8=6 measures ~1.65e-2 against the 2e-2 gate. DR MMs are placed
    contiguously at the head of each m-tile (mixed_c ordering measured
    1.06us/rep cheaper than interleaving them among the bf16 groups).
    Full fp8 digit schemes were measured and rejected: exact int8-weight
    arithmetic needs >= 3 fp8 digit-planes per k-block = 444 ns vs
    bf16's 2x220.
  - scale is folded into w at preproc (w_sc = bf16(w_q)*scale), so the
    psum evict is a single DVE bias-add.
  - w preproc: int32 staged [128,1024] chunks (loads split across the
    sync and gpsimd rings), DVE tensor_scalar_mul to bf16, SBUF->SBUF
    xbar transpose per 128-row strip into k-major banks: blocks
    NF8..31 into resident bf16 banks, blocks 0..NF8-1 into a bf16
    scratch then DVE-cast to the resident fp8 banks (the xbar transpose
    only supports 2-byte dtypes).
  - x: on-chip path (no DRAM round trip): fp32 [128,1024] chunk loads
    (gpsimd), DVE cast to a bf16 strip, SBUF->SBUF xbar transpose into
    k-major xT tiles; blocks 0..NF8-1 then DVE-cast to fp8 xT8. mt0's
    loads lead the gpsimd ring; startup transposes ride sync, steady
    -state ones the scalar ring behind the 16 one-time w transposes.
"""

import sys

if "/opt/trn_rl_repo" not in sys.path:
    sys.path.insert(0, "/opt/trn_rl_repo")

import numpy as np

B, S, IN_F, OUT_F = 4, 2048, 4096, 16384
NCORES = 8
O_SHARD = OUT_F // NCORES  # 2048
M_FULL = B * S  # 8192
NF8 = 6  # k-blocks (x128) computed in fp8 DoubleRow; must be even


def build_kernel_v4(nc, tc, M, K, O):
    import concourse.mybir as mybir

    f32 = mybir.dt.float32
    bf16 = mybir.dt.bfloat16
    f8 = mybir.dt.float8e4
    i32 = mybir.dt.int32
    DR = mybir.MatmulPerfMode.DoubleRow

    assert M % 128 == 0 and K % 1024 == 0 and O % 512 == 0
    KT = K // 128  # 32 contraction tiles
    MT = M // 128  # 64 m tiles
    NB = O // 512  # 4 psum-bank columns
    OT = O // 128  # 16 w row-chunks
    WCH = 1024  # w free-dim chunk for staging
    NWQ = K // WCH  # 4 chunks per ot
    XCH = 1024
    NXQ = K // XCH
    NBF = KT - NF8  # bf16 k-blocks
    assert NF8 % 2 == 0 and NBF % 2 == 0

    x_d = nc.dram_tensor("x", [M, K], f32, kind="ExternalInput").ap()
    w_d = nc.dram_tensor("w_q", [O, K], i32, kind="ExternalInput").ap()
    scale_d = nc.dram_tensor("scale", [1], f32, kind="ExternalInput").ap()
    bias_d = nc.dram_tensor("bias", [O], f32, kind="ExternalInput").ap()
    y_d = nc.dram_tensor("y", [M, O], f32, kind="ExternalOutput").ap()

    from contextlib import ExitStack

    ctx = ExitStack()
    tc_pool = lambda **kw: ctx.enter_context(tc.tile_pool(**kw))

    consts = tc_pool(name="consts", bufs=1)
    wq_pool = tc_pool(name="wq", bufs=1)
    wtmp_pool = tc_pool(name="wtmp", bufs=2)
    xs_pool = tc_pool(name="xs", bufs=1)  # 2 tags -> still double-buffered
    xbf_pool = tc_pool(name="xbf", bufs=1)
    xt_pool = tc_pool(name="xt", bufs=4)
    out_pool = tc_pool(name="outsb", bufs=4)
    psum_pool = tc_pool(name="psum", bufs=2, space="PSUM")

    # ---- constants ----
    scale128 = consts.tile([128, 1], f32, tag="scale128")
    nc.sync.dma_start(scale128[:], scale_d[None, :].partition_broadcast(128))

    # ---- x pipeline: fp32 chunk loads -> DVE cast -> xbar transpose ----
    def emit_xpipe(mt, load_eng, t_eng):
        xbf = xbf_pool.tile([128, K], bf16, tag="xbf", name=f"xbf{mt}")
        for q in range(NXQ):
            c0 = q * XCH
            xs = xs_pool.tile([128, XCH], f32, tag=f"xs{q % 2}", name=f"xs{mt}_{q}")
            load_eng.dma_start(xs[:], x_d[mt * 128 : (mt + 1) * 128, c0 : c0 + XCH])
            nc.vector.tensor_scalar_mul(xbf[:, c0 : c0 + XCH], xs[:], 1.0)
        xT = xt_pool.tile([128, KT, 128], bf16, tag="xT", name=f"xT{mt}")
        t_eng.dma_start(xT[:], xbf[:], transpose=True)
        if NF8:
            xT8 = xt_pool.tile([128, NF8, 128], f8, tag="xT8", name=f"xT8_{mt}")
            nc.vector.tensor_scalar_mul(xT8[:], xT[:, 0:NF8, :], 1.0)
        else:
            xT8 = None
        return xT, xT8

    # mt0 x loads lead the gpsimd ring (before the w q2/q3 chunks)
    xt_q = [emit_xpipe(0, nc.gpsimd, nc.sync)]

    # ---- w preproc ----
    # bf16 banks hold k-blocks NF8..31: wT_banks[b][p, kk, j] =
    #   w_sc[b*512 + j, (kk+NF8)*128 + p]
    # fp8 banks hold k-blocks 0..NF8-1: wT8_banks[b][p, k, j]
    wT_banks = [
        consts.tile([128, NBF, 512], bf16, tag=f"wT{b}", name=f"wT{b}")
        for b in range(NB)
    ]
    wT8_banks = [
        consts.tile([128, NF8, 512], f8, tag=f"wT8{b}", name=f"wT8{b}")
        for b in range(NB)
    ] if NF8 else []
    for ot in range(OT):
        r0 = ot * 128
        w_sc4 = wq_pool.tile([128, K], bf16, tag="wsc4", name=f"wsc{ot}")
        for q in range(NWQ):
            c0 = q * WCH
            w_stage = wq_pool.tile(
                [128, WCH], i32, tag=f"wstage{q % 2}", name=f"wst{ot}_{q}"
            )
            # split the 33.5MB of w loads across the sync + gpsimd rings
            eng = nc.sync if q < 2 else nc.gpsimd
            eng.dma_start(w_stage[:], w_d[r0 : r0 + 128, c0 : c0 + WCH])
            nc.vector.tensor_scalar_mul(
                w_sc4[:, c0 : c0 + WCH], w_stage[:], scale128[:, 0:1]
            )
        b, col = ot // 4, (ot % 4) * 128
        nc.scalar.dma_start(
            wT_banks[b][:, :, col : col + 128],
            w_sc4[:, NF8 * 128 :],
            transpose=True,
        )
        if NF8:
            wtmp = wtmp_pool.tile([128, NF8, 128], bf16, tag="wtmp", name=f"wtm{ot}")
            nc.scalar.dma_start(wtmp[:], w_sc4[:, 0 : NF8 * 128], transpose=True)
            nc.vector.tensor_scalar_mul(
                wT8_banks[b][:, :, col : col + 128], wtmp[:], 1.0
            )
        if ot == 0:
            # mt1 x loads follow bank0's first strip on gpsimd; transpose on
            # sync (scalar carries the w transposes)
            xt_q.append(emit_xpipe(1, nc.gpsimd, nc.sync))
        elif ot == 7:
            xt_q.append(emit_xpipe(2, nc.gpsimd, nc.sync))

    bias_bcast = consts.tile([128, O], f32, tag="bias_bcast")
    nc.sync.dma_start(bias_bcast[:], bias_d[None, :].partition_broadcast(128))

    # ---- main loop ----
    for mt in range(MT):
        if xt_q:
            xT, xT8 = xt_q.pop(0)
        else:
            xT, xT8 = emit_xpipe(mt, nc.gpsimd, nc.scalar)
        if mt + 3 < MT:
            xt_q.append(emit_xpipe(mt + 3, nc.gpsimd, nc.scalar))
        banks = [
            psum_pool.tile([128, 512], f32, tag=f"ps{b}", name=f"ps{mt}_{b}")
            for b in range(NB)
        ]
        if mt < 1:
            # staged start: bank-sequential so mt0 begins once bank 0's w
            # strips are ready, instead of waiting for all 16 (by mt1 all
            # banks have arrived, so only mt0 needs staging)
            for ob in range(NB):
                for t in range(NF8 // 2):
                    nc.tensor.matmul(
                        banks[ob][:],
                        xT8[:, 2 * t : 2 * t + 2, :],
                        wT8_banks[ob][:, 2 * t : 2 * t + 2, :],
                        start=(t == 0),
                        stop=False,
                        perf_mode=DR,
                    )
                for kk in range(NBF):
                    nc.tensor.matmul(
                        banks[ob][:],
                        xT[:, NF8 + kk, :],
                        wT_banks[ob][:, kk, :],
                        start=(NF8 == 0 and kk == 0),
                        stop=(kk == NBF - 1),
                    )
        else:
            # fp8 DR head (contiguous, t-outer: same-stationary x4 measured
            # faster than alternating for DR), then bf16 pairk over NF8..31
            for t in range(NF8 // 2):
                for ob in range(NB):
                    nc.tensor.matmul(
                        banks[ob][:],
                        xT8[:, 2 * t : 2 * t + 2, :],
                        wT8_banks[ob][:, 2 * t : 2 * t + 2, :],
                        start=(t == 0),
                        stop=False,
                        perf_mode=DR,
                    )
            for p in range(NBF // 2):
                for half in range(2):
                    for ob in range(NB):
                        kk = 2 * p + ((ob + half) % 2)
                        nc.tensor.matmul(
                            banks[ob][:],
                            xT[:, NF8 + kk, :],
                            wT_banks[ob][:, kk, :],
                            start=(NF8 == 0 and p == 0 and half == 0),
                            stop=(p == NBF // 2 - 1 and half == 1),
                        )
        for ob in range(NB):
            out_sb = out_pool.tile([128, 512], f32, tag="out", name=f"out{mt}_{ob}")
            nc.vector.tensor_add(
                out_sb[:], banks[ob][:], bias_bcast[:, ob * 512 : (ob + 1) * 512]
            )
            nc.sync.dma_start(
                y_d[mt * 128 : (mt + 1) * 128, ob * 512 : (ob + 1) * 512], out_sb[:]
            )

    ctx.close()


_CACHED_NC = None
LAST_RESULT = None


def _build_full_nc():
    global _CACHED_NC
    if _CACHED_NC is not None:
        return _CACHED_NC
    import concourse.tile as tile
    from concourse import bacc

    nc = bacc.Bacc(
        "TRN2",
        target_bir_lowering=False,
        debug=False,
        num_devices=NCORES,
    )
    with tile.TileContext(nc) as tc:
        build_kernel_v4(nc, tc, M_FULL, IN_F, O_SHARD)
    nc.compile()
    _CACHED_NC = nc
    return nc


def kernel(x, w_q, scale, bias):
    """Full inputs in, full output out. Shards w_q/bias over 8 cores."""
    from concourse.bass_utils import run_bass_kernel_spmd

    nc = _build_full_nc()

    x2 = np.ascontiguousarray(np.asarray(x, dtype=np.float32).reshape(M_FULL, IN_F))
    w2 = np.ascontiguousarray(np.asarray(w_q, dtype=np.int32))
    sc = np.asarray(scale, dtype=np.float32).reshape(1)
    bi = np.asarray(bias, dtype=np.float32)

    in_maps = []
    for c in range(NCORES):
        o0 = c * O_SHARD
        in_maps.append(
            {
                "x": x2,
                "w_q": np.ascontiguousarray(w2[o0 : o0 + O_SHARD]),
                "scale": sc,
                "bias": np.ascontiguousarray(bi[o0 : o0 + O_SHARD]),
            }
        )

    res = run_bass_kernel_spmd(nc, in_maps, core_ids=list(range(NCORES)))
    global LAST_RESULT
    LAST_RESULT = res
    shards = [res.results[c]["y"] for c in range(NCORES)]
    y = np.concatenate(shards, axis=1).reshape(B, S, OUT_F)
    return y.astype(np.float32)
